# revision 35
# baseline (speedup 1.0000x reference)
"""Multi-head attention (B=2, S=2048, E=1024, H=16) on 8 TRN2 NeuronCores.

Sharding: batch x head-group. Core c handles batch b=c//4 and head group
g=c%4 (4 heads = 256 of E). Each core computes its heads' attention output
slice and a partial fc_out product [S, E]; the host sums the 4 partials per
batch and adds b_out.

Device-side math per core:
  qpT = (Wq_g @ q[b].T + bq)      [256, S]   f32r (dims on partitions)
  kpT = (Wk_g @ k_c[b].T + bk)    [256, SKV] f32r (k compressed by mask)
  vp  = (v_c[b] @ Wv_g.T + bv)*m  [SKV, 4*65] bf16 (64 dims + ones col/head)
  per (qb, pt): S_T chunks [128kv, 512q] for both heads of the pair land in
  one 2-bank psum tile -> one Exp activation [128, 1024] -> et bf16; AV
  accumulates vp_aug.T @ E_T -> [65, 512] (row 64 = softmax denominator).
  normalize: Pool copies AV psum -> bf16; a [1,64] ones matmul broadcasts
  the denominator row; DVE reciprocal + multiply -> o_bf f16.
  fc_out accumulates BOTH pairs into one psum tile -> out [S, E] f16.

The loop is qb-major with fc(qb-1) and proj_q(qb+1) matmuls interleaved
into the attention stream so the PE never idles (p-state stays ramped).

Mask handling is exact: masked K/V rows are removed on the host (gather),
so softmax(where(mask==0, -1e20, e)) == exp(e_valid)/sum(exp(e_valid)).
"""

import os

import numpy as np

B, S, E, H = 2, 2048, 1024, 16
D = E // H           # 64
NCORES = 8
GROUPS = 4           # head groups per batch (cores per batch)
HPG = H // GROUPS    # 4 heads per core
DC = E // GROUPS     # 256 dims per core
NB = E // 128        # 8 contraction chunks over E
QB = 512             # query block
NQB = S // QB        # 4

_CACHE = {}


def _split_excess_waits(nc, max_waits=1):
    """walrus rejects instructions carrying >1 sem wait; spread extras onto
    single-wait NoOps inserted before the instruction on the same engine."""
    import concourse.mybir as mybir

    n_split = 0
    for f in nc.m.functions:
        for bb in f.blocks:
            out, changed = [], False
            for ins in bb.instructions:
                si = ins.sync_info
                if si is not None and si.on_wait is not None and len(si.on_wait) > max_waits:
                    waits = list(si.on_wait)
                    for w in waits[:-max_waits]:
                        out.append(mybir.InstNoOp(
                            name=nc.get_next_instruction_name(),
                            engine=ins.engine, ins=[], outs=[],
                            sync_info=mybir.SyncInfo(on_wait=[w], on_update=[])))
                        n_split += 1
                    ins.sync_info = mybir.SyncInfo(
                        on_wait=waits[-max_waits:], on_update=list(si.on_update))
                    changed = True
                out.append(ins)
            if changed:
                bb.instructions = out
    return n_split


def _build(skv, split_waits=True):
    import concourse.bass as bass
    import concourse.mybir as mybir
    import concourse.tile as tile

    f32 = mybir.dt.float32
    f32r = mybir.dt.float32r
    f16 = mybir.dt.float16
    bf16 = mybir.dt.bfloat16
    Alu = mybir.AluOpType
    Act = mybir.ActivationFunctionType

    nsk = skv // 128

    KCH = 384 if skv % 384 == 0 else 128
    nkch = skv // KCH

    nc = bass.Bass()
    # inputs pre-packed on host so each DMA is 128 partitions x one fat
    # contiguous run (multi-KB descriptors -> near-peak HBM bandwidth)
    xq_d = nc.declare_dram_parameter("xq", [S // QB, 128, NB, QB], f16, isOutput=False)
    xk_d = nc.declare_dram_parameter("xk", [nkch, 128, NB, KCH], f16, isOutput=False)
    xv_d = nc.declare_dram_parameter("xv", [nsk, 128, NB, 128], f16, isOutput=False)
    wqT = nc.declare_dram_parameter("wqT", [E, DC], f16, isOutput=False)
    wkT = nc.declare_dram_parameter("wkT", [E, DC], f16, isOutput=False)
    wvT = nc.declare_dram_parameter("wvT", [E, DC], f16, isOutput=False)
    woT = nc.declare_dram_parameter("woT", [DC, E], f16, isOutput=False)
    bq_d = nc.declare_dram_parameter("bq", [DC], f32, isOutput=False)
    bk_d = nc.declare_dram_parameter("bk", [DC], f32, isOutput=False)
    bv_d = nc.declare_dram_parameter("bv", [DC], f32, isOutput=False)
    vm_d = nc.declare_dram_parameter("vmask", [skv], f32, isOutput=False)
    sel_d = nc.declare_dram_parameter("sel", [2, 128], bf16, isOutput=False)
    out_d = nc.declare_dram_parameter("out", [S, E], f16, isOutput=True)

    with tile.TileContext(nc) as tc:
        with (
            tc.tile_pool(name="weights", bufs=4) as wpool,
            tc.tile_pool(name="consts", bufs=1) as cpool,
            tc.tile_pool(name="persist", bufs=1) as ppool,
            tc.tile_pool(name="xq", bufs=2) as xqpool,
            tc.tile_pool(name="xkv", bufs=1) as xkvpool,
            tc.tile_pool(name="sc_ps", bufs=2, space="PSUM") as scps,
            tc.tile_pool(name="av_ps", bufs=2, space="PSUM") as avps,
            tc.tile_pool(name="fc_ps", bufs=2, space="PSUM") as fps,
            tc.tile_pool(name="et", bufs=3) as etpool,
            tc.tile_pool(name="ou", bufs=3) as oupool,
            tc.tile_pool(name="rec", bufs=3) as rcpool,
            tc.tile_pool(name="outp", bufs=2) as opool,
            tc.tile_pool(name="small", bufs=2) as smpool,
        ):
            # ---- constants / weights. All big inputs go on the sync queue
            # in priority order (its FIFO then orders the transfers on the
            # bandwidth-saturated DMA engines); consts on gpsimd; scalar
            # queue stays pure-exp. k first: kpT gates attention.
            wk_t = wpool.tile([128, NB, DC], f16, tag="w", name="wk_t")
            wq_t = wpool.tile([128, NB, DC], f16, tag="w", name="wq_t")
            wv_t = wpool.tile([128, NB, DC], f16, tag="w", name="wv_t")
            wo_t = wpool.tile([128, DC // 128, E], f16, tag="w", name="wo_t")
            nc.sync.dma_start(wk_t[:], wkT.rearrange("(ko p) m -> p ko m", p=128))

            xk = xkvpool.tile([128, nkch, NB, KCH], f16, tag="xk", name="xk")
            xv = xkvpool.tile([128, nsk, NB, 128], f16, tag="xv", name="xv")
            for kb in range(nkch):
                nc.sync.dma_start(xk[:, kb, :, :], xk_d[kb])

            xq = [xqpool.tile([128, NB, QB], f16, tag="xq", name=f"xq{nb}")
                  for nb in range(NQB)]
            nc.sync.dma_start(wq_t[:], wqT.rearrange("(ko p) m -> p ko m", p=128))
            nc.sync.dma_start(xq[0][:], xq_d[0])
            nc.sync.dma_start(wv_t[:], wvT.rearrange("(ko p) m -> p ko m", p=128))
            for sc in range(nsk):
                nc.sync.dma_start(xv[:, sc, :, :], xv_d[sc])
            for nb in range(1, NQB):
                nc.sync.dma_start(xq[nb][:], xq_d[nb])
            nc.sync.dma_start(wo_t[:], woT.rearrange("(ko p) n -> p ko n", p=128))

            bq_t = cpool.tile([128, 2], f32, tag="bq")
            bk_t = cpool.tile([128, 2], f32, tag="bk")
            bv_t = cpool.tile([128, DC], f32, tag="bv")
            vm_t = cpool.tile([128, nsk], f32, tag="vm")
            nc.gpsimd.dma_start(bk_t[:], bk_d.rearrange("(c p) -> p c", p=128))
            nc.gpsimd.dma_start(bq_t[:], bq_d.rearrange("(c p) -> p c", p=128))
            nc.gpsimd.dma_start(bv_t[:], bv_d[None, :].to_broadcast((128, DC)))
            nc.gpsimd.dma_start(vm_t[:], vm_d.rearrange("(s p) -> p s", p=128))

            # selection matrix: sel[k, p] = 1 iff p // 64 == k; broadcasts
            # rr row j to output partitions 64j..64j+63 in one matmul
            sel_t = cpool.tile([2, 128], bf16, tag="sel")
            nc.gpsimd.dma_start(sel_t[:], sel_d[:])

            qpT = ppool.tile([128, 2, S], f32r, tag="qpT")
            kpT = ppool.tile([128, 2, skv], f32r, tag="kpT")
            vp = ppool.tile([128, nsk, HPG, D + 1], bf16, tag="vp")
            o_bf = ppool.tile([128, 2, S], f16, tag="o_bf")

            # ---- projection emitters ----
            def proj_k(kb):
                off = kb * KCH
                for mc in range(2):
                    ps = fps.tile([128, 512], f32, tag="fc", name="kp_ps")[:, :KCH]
                    for kc in range(NB):
                        nc.tensor.matmul(
                            ps[:], wk_t[:, kc, mc * 128:(mc + 1) * 128],
                            xk[:, kb, kc, :],
                            start=(kc == 0), stop=(kc == NB - 1))
                    nc.vector.tensor_tensor(
                        out=kpT[:, mc, off:off + KCH], in0=ps[:],
                        in1=bk_t[:, mc:mc + 1].to_broadcast((128, KCH)), op=Alu.add)

            def proj_q_mms(nb, mc, ps):
                """8 accumulating matmuls for qpT block nb, half mc."""
                for kc in range(NB):
                    yield lambda kc=kc: nc.tensor.matmul(
                        ps[:], wq_t[:, kc, mc * 128:(mc + 1) * 128],
                        xq[nb][:, kc, :], start=(kc == 0), stop=(kc == NB - 1))

            def proj_q_bias(nb, mc, ps):
                nc.vector.tensor_tensor(
                    out=qpT[:, mc, nb * QB:(nb + 1) * QB], in0=ps[:],
                    in1=bq_t[:, mc:mc + 1].to_broadcast((128, QB)), op=Alu.add)

            def proj_q(nb):
                for mc in range(2):
                    ps = fps.tile([128, 512], f32, tag="fc", name="qp_ps")
                    for mm in proj_q_mms(nb, mc, ps):
                        mm()
                    proj_q_bias(nb, mc, ps)

            def proj_v(sc):
                ps = fps.tile([128, 512], f32, tag="fc", name="vp_ps")[:, :DC]
                for kc in range(NB):
                    nc.tensor.matmul(
                        ps[:], xv[:, sc, kc, :], wv_t[:, kc, :],
                        start=(kc == 0), stop=(kc == NB - 1))
                t1 = smpool.tile([128, DC], f32, tag="vtmp")
                nc.vector.tensor_tensor(out=t1[:], in0=ps[:], in1=bv_t[:], op=Alu.add)
                nc.gpsimd.tensor_tensor(
                    out=vp[:, sc, :, 0:D],
                    in0=t1.rearrange("p (h w) -> p h w", w=D),
                    in1=vm_t[:, sc:sc + 1, None].to_broadcast((128, HPG, D)),
                    op=Alu.mult)
                nc.gpsimd.tensor_copy(
                    out=vp[:, sc, :, D:D + 1],
                    in_=vm_t[:, sc:sc + 1, None].to_broadcast((128, HPG, 1)))

            # ---- lead-in: kpT, qpT block 0 (vp is projected just-in-time
            # inside qb0/pt0's chunk loop as xv chunks stream in) ----
            for kb in range(nkch):
                proj_k(kb)
            proj_q(0)

            # ---- fc_out emitter (query row sqc: 128 queries, eb: 512 cols) --
            def fc_mms(sqc, eb, ps):
                for pt in range(2):
                    yield lambda pt=pt: nc.tensor.matmul(
                        ps[:], o_bf[:, pt, sqc * 128:(sqc + 1) * 128],
                        wo_t[:, pt, eb * 512:(eb + 1) * 512],
                        start=(pt == 0), stop=(pt == 1))

            def fc_tail(sqc, eb, ps, ob, last=False):
                if last and eb == 0:
                    nc.scalar.activation(ob[:, 0:512], ps[:], Act.Copy)
                else:
                    nc.vector.tensor_copy(
                        out=ob[:, eb * 512:(eb + 1) * 512], in_=ps[:])
                if eb == 1:
                    eng = nc.scalar if sqc % 2 else nc.sync
                    eng.dma_start(out_d[sqc * 128:(sqc + 1) * 128, :], ob[:])

            def fc_block(qb, last=False):
                """All fc work for query block qb as a list of thunks
                (each one PE matmul or a tail), to interleave."""
                thunks = []
                for sq in range(QB // 128):
                    sqc = qb * (QB // 128) + sq
                    ob = opool.tile([128, E], f16, tag="ob", name="ob")
                    for eb in range(2):
                        ps = fps.tile([128, 512], f32, tag="fc", name="fc_ps")
                        for mm in fc_mms(sqc, eb, ps):
                            thunks.append(mm)
                        thunks.append(
                            lambda sqc=sqc, eb=eb, ps=ps, ob=ob:
                                fc_tail(sqc, eb, ps, ob, last))
                return thunks

            # ---- attention ----
            for qb in range(NQB):
                q0 = qb * QB
                # interleavable PE work: fc of previous block + proj of next q
                extra = fc_block(qb - 1) if qb > 0 else []
                if qb + 1 < NQB:
                    for mc in range(2):
                        ps = fps.tile([128, 512], f32, tag="fc", name="qp_ps")
                        for mm in proj_q_mms(qb + 1, mc, ps):
                            extra.append(mm)
                        extra.append(
                            lambda nb=qb + 1, mc=mc, ps=ps: proj_q_bias(nb, mc, ps))
                ei = 0

                def drain(n):
                    nonlocal ei
                    for _ in range(n):
                        if ei < len(extra):
                            extra[ei]()
                            ei += 1

                for pt in range(2):
                    ps_av = [avps.tile([D + 1, QB], f32, tag="av", name=f"av{j}")
                             for j in range(2)]
                    ets = []
                    for skc in range(nsk):
                        psx = scps.tile([128, 2, QB], f32, tag="sc", name="psx")
                        for j in range(2):
                            nc.tensor.matmul(
                                psx[:, j, :],
                                kpT[64 * j:64 * j + 64, pt, skc * 128:(skc + 1) * 128],
                                qpT[64 * j:64 * j + 64, pt, q0:q0 + QB],
                                start=True, stop=True, tile_position=(64 * j, 0))
                        if qb == 0 and pt == 0:
                            proj_v(skc)  # just-in-time: AV(skc) runs next step
                        et = etpool.tile([128, 2, QB], bf16, tag="et", name="et")
                        nc.scalar.activation(et[:], psx[:], Act.Exp)
                        ets.append(et)
                        # AV for previous chunk (keeps one step of slack
                        # between PE and the exp on the scalar engine)
                        if skc > 0:
                            for j in range(2):
                                nc.tensor.matmul(
                                    ps_av[j][:], vp[:, skc - 1, 2 * pt + j, :],
                                    ets[skc - 1][:, j, :],
                                    start=(skc - 1 == 0), stop=False)
                        if not (qb == 0 and pt == 0):
                            drain(2)
                    for j in range(2):
                        nc.tensor.matmul(
                            ps_av[j][:], vp[:, nsk - 1, 2 * pt + j, :],
                            ets[nsk - 1][:, j, :],
                            start=False, stop=True)
                    # normalize: Act drains AV psum to bf16; denominator rows
                    # DMA-reshaped to [128, 8] for a cheap reciprocal, then
                    # broadcast back via a [1,64] ones matmul.
                    ous = []
                    scol = rcpool.tile([128, 8], bf16, tag="scol", name="scol")
                    for j in range(2):
                        ou = oupool.tile([D + 1, QB], bf16, tag="ou", name="ou")
                        nc.vector.tensor_copy(out=ou[:], in_=ps_av[j][:])
                        ous.append(ou)
                        # denom row q=p*4+c -> scol[p, j*4+c]
                        nc.gpsimd.dma_start(
                            scol[:, j * 4:(j + 1) * 4],
                            ou[64:65, :].rearrange("o (p c) -> o p c", c=4))
                    with nc.allow_low_precision(reason="softmax denom recip"):
                        nc.vector.reciprocal(out=scol[:], in_=scol[:])
                    rr = rcpool.tile([2, QB], bf16, tag="rr", name="rr")
                    # scol[p, j*4+c] -> rr[j, p*4+c], iterating (p, c)
                    for j in range(2):
                        nc.gpsimd.dma_start(
                            rr[j:j + 1, :].rearrange("o (p c) -> o p c", c=4),
                            scol[:, j * 4:(j + 1) * 4])
                    bc = fps.tile([128, 512], f32, tag="fc", name="bc")
                    nc.tensor.matmul(bc[:], sel_t[:], rr[:],
                                     start=True, stop=True)
                    for j in range(2):
                        nc.vector.tensor_tensor(
                            out=o_bf[64 * j:64 * j + 64, pt, q0:q0 + QB],
                            in0=ous[j][0:D, :],
                            in1=bc[64 * j:64 * j + 64, :], op=Alu.mult)
                    drain(4)
                # any leftover interleaved work
                drain(len(extra))
            # final fc block
            for th in fc_block(NQB - 1, last=True):
                th()

    if split_waits:
        _split_excess_waits(nc)
    return nc


def _prep_inputs(q, k, v, mask, W_qkv, b_qkv, W_out, b_out):
    """Host-side shard/layout prep. Returns (skv, in_maps)."""
    q = np.asarray(q, dtype=np.float32)
    k = np.asarray(k, dtype=np.float32)
    v = np.asarray(v, dtype=np.float32)
    mask = np.asarray(mask)
    W_qkv = np.asarray(W_qkv, dtype=np.float32)
    b_qkv = np.asarray(b_qkv, dtype=np.float32)
    W_out = np.asarray(W_out, dtype=np.float32)

    valid = [np.nonzero(mask[b, 0, 0] != 0)[0] for b in range(B)]
    cnts = [len(vi) for vi in valid]
    skv = max(128, max((c + 127) // 128 * 128 for c in cnts))
    KCH = 384 if skv % 384 == 0 else 128
    nkch = skv // KCH
    nsk = skv // 128

    # pack [E, n] tensors as [chunk, 128, NB, w]: per-(chunk, partition)
    # contiguous runs so each device DMA is 128 fat descriptors
    def pack(tT, w):
        n = tT.shape[1]
        return np.ascontiguousarray(
            tT.reshape(NB, 128, n // w, w).transpose(2, 1, 0, 3))

    qT, kTc, vTc, vms = [], [], [], []
    for b in range(B):
        qT.append(pack(q[b].T.astype(np.float16), 512))
        kt = np.zeros((E, skv), np.float16)
        vt = np.zeros((E, skv), np.float16)
        kt[:, :cnts[b]] = k[b][valid[b]].T
        vt[:, :cnts[b]] = v[b][valid[b]].T
        kTc.append(pack(kt, KCH))
        vTc.append(pack(vt, 128))
        vm = np.zeros((skv,), np.float32)
        vm[:cnts[b]] = 1.0
        vms.append(vm)

    import ml_dtypes
    sel = np.zeros((2, 128), dtype=ml_dtypes.bfloat16)
    sel[0, 0:64] = 1
    sel[1, 64:128] = 1

    in_maps = []
    for c in range(NCORES):
        b, g = divmod(c, GROUPS)
        sl = slice(g * DC, (g + 1) * DC)
        in_maps.append({
            "xq": qT[b], "xk": kTc[b], "xv": vTc[b],
            "wqT": np.ascontiguousarray(W_qkv[sl, :].T).astype(np.float16),
            "wkT": np.ascontiguousarray(W_qkv[E:][sl, :].T).astype(np.float16),
            "wvT": np.ascontiguousarray(W_qkv[2 * E:][sl, :].T).astype(np.float16),
            "woT": np.ascontiguousarray(W_out[:, sl].T).astype(np.float16),
            "bq": np.ascontiguousarray(b_qkv[sl]),
            "bk": np.ascontiguousarray(b_qkv[E:][sl]),
            "bv": np.ascontiguousarray(b_qkv[2 * E:][sl]),
            "vmask": vms[b],
            "sel": sel,
        })
    return skv, in_maps


def kernel(q, k, v, mask, W_qkv, b_qkv, W_out, b_out):
    from concourse import bass_utils

    skv, in_maps = _prep_inputs(q, k, v, mask, W_qkv, b_qkv, W_out, b_out)
    if skv not in _CACHE:
        _CACHE[skv] = _build(skv)
    nc = _CACHE[skv]

    trace = os.environ.get("KERNEL_TRACE") == "1"
    if trace:
        bass_utils.upload_artifacts = lambda tmpdir: "local://" + tmpdir
    res = bass_utils.run_bass_kernel_spmd(
        nc, in_maps, list(range(NCORES)), trace=trace)
    if trace:
        global LAST_RES
        LAST_RES = res
        print(f"HW exec time: {res.exec_time_ns} ns")

    b_out = np.asarray(b_out, dtype=np.float32)
    out = np.zeros((B, S, E), np.float32)
    for c in range(NCORES):
        out[c // GROUPS] += res.results[c]["out"].astype(np.float32)
    out += b_out[None, None, :]
    return out


# revision 36
# speedup vs baseline: 1.1023x; 1.1023x over previous
"""Multi-head attention (B=2, S=2048, E=1024, H=16) on 8 TRN2 NeuronCores.

Sharding: batch x head-group. Core c handles batch b=c//4 and head group
g=c%4 (4 heads = 256 of E). Each core computes its heads' attention output
slice and a partial fc_out product [S, E]; the host sums the 4 partials per
batch and adds b_out.

Device-side math per core:
  qpT = (Wq_g @ q[b].T + bq)      [256, S]   f32r (dims on partitions)
  kpT = (Wk_g @ k_c[b].T + bk)    [256, SKV] f32r (k compressed by mask)
  vp  = (v_c[b] @ Wv_g.T + bv)*m  [SKV, 4*65] bf16 (64 dims + ones col/head)
  per (qb, pt): S_T chunks [128kv, 512q] for both heads of the pair land in
  one 2-bank psum tile -> one Exp activation [128, 1024] -> et bf16; AV
  accumulates vp_aug.T @ E_T -> [65, 512] (row 64 = softmax denominator).
  normalize: Pool copies AV psum -> bf16; a [1,64] ones matmul broadcasts
  the denominator row; DVE reciprocal + multiply -> o_bf f16.
  fc_out accumulates BOTH pairs into one psum tile -> out [S, E] f16.

The loop is qb-major with fc(qb-1) and proj_q(qb+1) matmuls interleaved
into the attention stream so the PE never idles (p-state stays ramped).

Mask handling is exact: masked K/V rows are removed on the host (gather),
so softmax(where(mask==0, -1e20, e)) == exp(e_valid)/sum(exp(e_valid)).
"""

import os

import numpy as np

B, S, E, H = 2, 2048, 1024, 16
D = E // H           # 64
NCORES = 8
GROUPS = 4           # head groups per batch (cores per batch)
HPG = H // GROUPS    # 4 heads per core
DC = E // GROUPS     # 256 dims per core
NB = E // 128        # 8 contraction chunks over E
QB = 512             # query block
NQB = S // QB        # 4

_CACHE = {}


def _split_excess_waits(nc, max_waits=1):
    """walrus rejects instructions carrying >1 sem wait; spread extras onto
    single-wait NoOps inserted before the instruction on the same engine."""
    import concourse.mybir as mybir

    n_split = 0
    for f in nc.m.functions:
        for bb in f.blocks:
            out, changed = [], False
            for ins in bb.instructions:
                si = ins.sync_info
                if si is not None and si.on_wait is not None and len(si.on_wait) > max_waits:
                    waits = list(si.on_wait)
                    for w in waits[:-max_waits]:
                        out.append(mybir.InstNoOp(
                            name=nc.get_next_instruction_name(),
                            engine=ins.engine, ins=[], outs=[],
                            sync_info=mybir.SyncInfo(on_wait=[w], on_update=[])))
                        n_split += 1
                    ins.sync_info = mybir.SyncInfo(
                        on_wait=waits[-max_waits:], on_update=list(si.on_update))
                    changed = True
                out.append(ins)
            if changed:
                bb.instructions = out
    return n_split


def _build(skv, split_waits=True):
    import concourse.bass as bass
    import concourse.mybir as mybir
    import concourse.tile as tile

    f32 = mybir.dt.float32
    f32r = mybir.dt.float32r
    f16 = mybir.dt.float16
    bf16 = mybir.dt.bfloat16
    Alu = mybir.AluOpType
    Act = mybir.ActivationFunctionType

    nsk = skv // 128

    KCH = 384 if skv % 384 == 0 else 128
    nkch = skv // KCH

    nc = bass.Bass()
    # inputs pre-packed on host so each DMA is 128 partitions x one fat
    # contiguous run (multi-KB descriptors -> near-peak HBM bandwidth)
    xq_d = nc.declare_dram_parameter("xq", [S // QB, 128, NB, QB], f16, isOutput=False)
    xk_d = nc.declare_dram_parameter("xk", [nkch, 128, NB, KCH], f16, isOutput=False)
    xv_d = nc.declare_dram_parameter("xv", [nsk, 128, NB, 128], f16, isOutput=False)
    wqT = nc.declare_dram_parameter("wqT", [E, DC], f16, isOutput=False)
    wkT = nc.declare_dram_parameter("wkT", [E, DC], f16, isOutput=False)
    wvT = nc.declare_dram_parameter("wvT", [E, DC], f16, isOutput=False)
    woT = nc.declare_dram_parameter("woT", [DC, E], f16, isOutput=False)
    bq_d = nc.declare_dram_parameter("bq", [DC], f32, isOutput=False)
    bk_d = nc.declare_dram_parameter("bk", [DC], f32, isOutput=False)
    bv_d = nc.declare_dram_parameter("bv", [DC], f32, isOutput=False)
    vm_d = nc.declare_dram_parameter("vmask", [skv], f32, isOutput=False)
    sel_d = nc.declare_dram_parameter("sel", [2, 128], bf16, isOutput=False)
    out_d = nc.declare_dram_parameter("out", [S, E], f16, isOutput=True)

    with tile.TileContext(nc) as tc:
        with (
            tc.tile_pool(name="weights", bufs=4) as wpool,
            tc.tile_pool(name="consts", bufs=1) as cpool,
            tc.tile_pool(name="persist", bufs=1) as ppool,
            tc.tile_pool(name="xq", bufs=2) as xqpool,
            tc.tile_pool(name="xkv", bufs=1) as xkvpool,
            tc.tile_pool(name="sc_ps", bufs=2, space="PSUM") as scps,
            tc.tile_pool(name="av_ps", bufs=2, space="PSUM") as avps,
            tc.tile_pool(name="fc_ps", bufs=2, space="PSUM") as fps,
            tc.tile_pool(name="et", bufs=3) as etpool,
            tc.tile_pool(name="ou", bufs=3) as oupool,
            tc.tile_pool(name="rec", bufs=3) as rcpool,
            tc.tile_pool(name="outp", bufs=2) as opool,
            tc.tile_pool(name="small", bufs=2) as smpool,
        ):
            # ---- constants / weights. All big inputs go on the sync queue
            # in priority order (its FIFO then orders the transfers on the
            # bandwidth-saturated DMA engines); consts on gpsimd; scalar
            # queue stays pure-exp. k first: kpT gates attention.
            wk_t = wpool.tile([128, NB, DC], f16, tag="w", name="wk_t")
            wq_t = wpool.tile([128, NB, DC], f16, tag="w", name="wq_t")
            wv_t = wpool.tile([128, NB, DC], f16, tag="w", name="wv_t")
            wo_t = wpool.tile([128, DC // 128, E], f16, tag="w", name="wo_t")
            nc.sync.dma_start(wk_t[:], wkT.rearrange("(ko p) m -> p ko m", p=128))

            xk = xkvpool.tile([128, nkch, NB, KCH], f16, tag="xk", name="xk")
            xv = xkvpool.tile([128, nsk, NB, 128], f16, tag="xv", name="xv")
            for kb in range(nkch):
                nc.sync.dma_start(xk[:, kb, :, :], xk_d[kb])

            xq = [xqpool.tile([128, NB, QB], f16, tag="xq", name=f"xq{nb}")
                  for nb in range(NQB)]
            nc.sync.dma_start(wq_t[:], wqT.rearrange("(ko p) m -> p ko m", p=128))
            nc.sync.dma_start(xq[0][:], xq_d[0])
            nc.sync.dma_start(wv_t[:], wvT.rearrange("(ko p) m -> p ko m", p=128))
            for sc in range(nsk):
                nc.sync.dma_start(xv[:, sc, :, :], xv_d[sc])
            for nb in range(1, NQB):
                nc.sync.dma_start(xq[nb][:], xq_d[nb])
            nc.sync.dma_start(wo_t[:], woT.rearrange("(ko p) n -> p ko n", p=128))

            bq_t = cpool.tile([128, 2], f32, tag="bq")
            bk_t = cpool.tile([128, 2], f32, tag="bk")
            bv_t = cpool.tile([128, DC], f32, tag="bv")
            vm_t = cpool.tile([128, nsk], f32, tag="vm")
            nc.gpsimd.dma_start(bk_t[:], bk_d.rearrange("(c p) -> p c", p=128))
            nc.gpsimd.dma_start(bq_t[:], bq_d.rearrange("(c p) -> p c", p=128))
            nc.gpsimd.dma_start(bv_t[:], bv_d[None, :].to_broadcast((128, DC)))
            nc.gpsimd.dma_start(vm_t[:], vm_d.rearrange("(s p) -> p s", p=128))

            # selection matrix: sel[k, p] = 1 iff p // 64 == k; broadcasts
            # rr row j to output partitions 64j..64j+63 in one matmul
            sel_t = cpool.tile([2, 128], bf16, tag="sel")
            nc.gpsimd.dma_start(sel_t[:], sel_d[:])

            qpT = ppool.tile([128, 2, S], f32r, tag="qpT")
            kpT = ppool.tile([128, 2, skv], f32r, tag="kpT")
            vp = ppool.tile([128, nsk, HPG, D + 1], bf16, tag="vp")
            o_bf = ppool.tile([128, 2, S], f16, tag="o_bf")

            # ---- projection emitters ----
            def proj_k(kb):
                off = kb * KCH
                for mc in range(2):
                    ps = fps.tile([128, 512], f32, tag="fc", name="kp_ps")[:, :KCH]
                    for kc in range(NB):
                        nc.tensor.matmul(
                            ps[:], wk_t[:, kc, mc * 128:(mc + 1) * 128],
                            xk[:, kb, kc, :],
                            start=(kc == 0), stop=(kc == NB - 1))
                    nc.vector.tensor_tensor(
                        out=kpT[:, mc, off:off + KCH], in0=ps[:],
                        in1=bk_t[:, mc:mc + 1].to_broadcast((128, KCH)), op=Alu.add)

            def proj_q_mms(nb, mc, ps):
                """8 accumulating matmuls for qpT block nb, half mc."""
                for kc in range(NB):
                    yield lambda kc=kc: nc.tensor.matmul(
                        ps[:], wq_t[:, kc, mc * 128:(mc + 1) * 128],
                        xq[nb][:, kc, :], start=(kc == 0), stop=(kc == NB - 1))

            def proj_q_bias(nb, mc, ps):
                nc.vector.tensor_tensor(
                    out=qpT[:, mc, nb * QB:(nb + 1) * QB], in0=ps[:],
                    in1=bq_t[:, mc:mc + 1].to_broadcast((128, QB)), op=Alu.add)

            def proj_q(nb):
                for mc in range(2):
                    ps = fps.tile([128, 512], f32, tag="fc", name="qp_ps")
                    for mm in proj_q_mms(nb, mc, ps):
                        mm()
                    proj_q_bias(nb, mc, ps)

            def proj_v(sc):
                ps = fps.tile([128, 512], f32, tag="fc", name="vp_ps")[:, :DC]
                for kc in range(NB):
                    nc.tensor.matmul(
                        ps[:], xv[:, sc, kc, :], wv_t[:, kc, :],
                        start=(kc == 0), stop=(kc == NB - 1))
                t1 = smpool.tile([128, DC], f32, tag="vtmp")
                nc.vector.tensor_tensor(out=t1[:], in0=ps[:], in1=bv_t[:], op=Alu.add)
                nc.gpsimd.tensor_tensor(
                    out=vp[:, sc, :, 0:D],
                    in0=t1.rearrange("p (h w) -> p h w", w=D),
                    in1=vm_t[:, sc:sc + 1, None].to_broadcast((128, HPG, D)),
                    op=Alu.mult)
                nc.gpsimd.tensor_copy(
                    out=vp[:, sc, :, D:D + 1],
                    in_=vm_t[:, sc:sc + 1, None].to_broadcast((128, HPG, 1)))

            # ---- lead-in: kpT, qpT block 0 (vp is projected just-in-time
            # inside qb0/pt0's chunk loop as xv chunks stream in) ----
            for kb in range(nkch):
                proj_k(kb)
            proj_q(0)

            # ---- fc_out emitter (query row sqc: 128 queries, eb: 512 cols) --
            def fc_mms(sqc, eb, ps):
                for pt in range(2):
                    yield lambda pt=pt: nc.tensor.matmul(
                        ps[:], o_bf[:, pt, sqc * 128:(sqc + 1) * 128],
                        wo_t[:, pt, eb * 512:(eb + 1) * 512],
                        start=(pt == 0), stop=(pt == 1))

            def fc_tail(sqc, eb, ps, ob, last=False):
                nc.vector.tensor_copy(
                    out=ob[:, eb * 512:(eb + 1) * 512], in_=ps[:])
                if eb == 1:
                    nc.sync.dma_start(out_d[sqc * 128:(sqc + 1) * 128, :], ob[:])

            def fc_block(qb, last=False):
                """All fc work for query block qb as a list of thunks
                (each one PE matmul or a tail), to interleave."""
                thunks = []
                for sq in range(QB // 128):
                    sqc = qb * (QB // 128) + sq
                    ob = opool.tile([128, E], f16, tag="ob", name="ob")
                    for eb in range(2):
                        ps = fps.tile([128, 512], f32, tag="fc", name="fc_ps")
                        for mm in fc_mms(sqc, eb, ps):
                            thunks.append(mm)
                        thunks.append(
                            lambda sqc=sqc, eb=eb, ps=ps, ob=ob:
                                fc_tail(sqc, eb, ps, ob, last))
                return thunks

            # ---- attention ----
            for qb in range(NQB):
                q0 = qb * QB
                # interleavable PE work: fc of previous block + proj of next q
                extra = fc_block(qb - 1) if qb > 0 else []
                if qb + 1 < NQB:
                    for mc in range(2):
                        ps = fps.tile([128, 512], f32, tag="fc", name="qp_ps")
                        for mm in proj_q_mms(qb + 1, mc, ps):
                            extra.append(mm)
                        extra.append(
                            lambda nb=qb + 1, mc=mc, ps=ps: proj_q_bias(nb, mc, ps))
                ei = 0

                def drain(n):
                    nonlocal ei
                    for _ in range(n):
                        if ei < len(extra):
                            extra[ei]()
                            ei += 1

                for pt in range(2):
                    ps_av = [avps.tile([D + 1, QB], f32, tag="av", name=f"av{j}")
                             for j in range(2)]
                    ets = []
                    for skc in range(nsk):
                        psx = scps.tile([128, 2, QB], f32, tag="sc", name="psx")
                        for j in range(2):
                            nc.tensor.matmul(
                                psx[:, j, :],
                                kpT[64 * j:64 * j + 64, pt, skc * 128:(skc + 1) * 128],
                                qpT[64 * j:64 * j + 64, pt, q0:q0 + QB],
                                start=True, stop=True, tile_position=(64 * j, 0))
                        if qb == 0 and pt == 0:
                            proj_v(skc)  # just-in-time: AV(skc) runs next step
                        et = etpool.tile([128, 2, QB], bf16, tag="et", name="et")
                        nc.scalar.activation(et[:], psx[:], Act.Exp)
                        ets.append(et)
                        # AV for previous chunk (keeps one step of slack
                        # between PE and the exp on the scalar engine)
                        if skc > 0:
                            for j in range(2):
                                nc.tensor.matmul(
                                    ps_av[j][:], vp[:, skc - 1, 2 * pt + j, :],
                                    ets[skc - 1][:, j, :],
                                    start=(skc - 1 == 0), stop=False)
                        if not (qb == 0 and pt == 0):
                            drain(2)
                    for j in range(2):
                        nc.tensor.matmul(
                            ps_av[j][:], vp[:, nsk - 1, 2 * pt + j, :],
                            ets[nsk - 1][:, j, :],
                            start=False, stop=True)
                    # normalize: Act drains AV psum to bf16; denominator rows
                    # DMA-reshaped to [128, 8] for a cheap reciprocal, then
                    # broadcast back via a [1,64] ones matmul.
                    ous = []
                    scol = rcpool.tile([128, 8], bf16, tag="scol", name="scol")
                    for j in range(2):
                        ou = oupool.tile([D + 1, QB], bf16, tag="ou", name="ou")
                        nc.vector.tensor_copy(out=ou[:], in_=ps_av[j][:])
                        ous.append(ou)
                        # denom row q=p*4+c -> scol[p, j*4+c]
                        nc.gpsimd.dma_start(
                            scol[:, j * 4:(j + 1) * 4],
                            ou[64:65, :].rearrange("o (p c) -> o p c", c=4))
                    with nc.allow_low_precision(reason="softmax denom recip"):
                        nc.vector.reciprocal(out=scol[:], in_=scol[:])
                    rr = rcpool.tile([2, QB], bf16, tag="rr", name="rr")
                    # scol[p, j*4+c] -> rr[j, p*4+c], iterating (p, c)
                    for j in range(2):
                        nc.gpsimd.dma_start(
                            rr[j:j + 1, :].rearrange("o (p c) -> o p c", c=4),
                            scol[:, j * 4:(j + 1) * 4])
                    bc = fps.tile([128, 512], f32, tag="fc", name="bc")
                    nc.tensor.matmul(bc[:], sel_t[:], rr[:],
                                     start=True, stop=True)
                    for j in range(2):
                        nc.vector.tensor_tensor(
                            out=o_bf[64 * j:64 * j + 64, pt, q0:q0 + QB],
                            in0=ous[j][0:D, :],
                            in1=bc[64 * j:64 * j + 64, :], op=Alu.mult)
                    drain(4)
                # any leftover interleaved work
                drain(len(extra))
            # final fc block
            for th in fc_block(NQB - 1, last=True):
                th()

    if split_waits:
        _split_excess_waits(nc)
    return nc


def _prep_inputs(q, k, v, mask, W_qkv, b_qkv, W_out, b_out):
    """Host-side shard/layout prep. Returns (skv, in_maps)."""
    q = np.asarray(q, dtype=np.float32)
    k = np.asarray(k, dtype=np.float32)
    v = np.asarray(v, dtype=np.float32)
    mask = np.asarray(mask)
    W_qkv = np.asarray(W_qkv, dtype=np.float32)
    b_qkv = np.asarray(b_qkv, dtype=np.float32)
    W_out = np.asarray(W_out, dtype=np.float32)

    valid = [np.nonzero(mask[b, 0, 0] != 0)[0] for b in range(B)]
    cnts = [len(vi) for vi in valid]
    skv = max(128, max((c + 127) // 128 * 128 for c in cnts))
    KCH = 384 if skv % 384 == 0 else 128
    nkch = skv // KCH
    nsk = skv // 128

    # pack [E, n] tensors as [chunk, 128, NB, w]: per-(chunk, partition)
    # contiguous runs so each device DMA is 128 fat descriptors
    def pack(tT, w):
        n = tT.shape[1]
        return np.ascontiguousarray(
            tT.reshape(NB, 128, n // w, w).transpose(2, 1, 0, 3))

    qT, kTc, vTc, vms = [], [], [], []
    for b in range(B):
        qT.append(pack(q[b].T.astype(np.float16), 512))
        kt = np.zeros((E, skv), np.float16)
        vt = np.zeros((E, skv), np.float16)
        kt[:, :cnts[b]] = k[b][valid[b]].T
        vt[:, :cnts[b]] = v[b][valid[b]].T
        kTc.append(pack(kt, KCH))
        vTc.append(pack(vt, 128))
        vm = np.zeros((skv,), np.float32)
        vm[:cnts[b]] = 1.0
        vms.append(vm)

    import ml_dtypes
    sel = np.zeros((2, 128), dtype=ml_dtypes.bfloat16)
    sel[0, 0:64] = 1
    sel[1, 64:128] = 1

    in_maps = []
    for c in range(NCORES):
        b, g = divmod(c, GROUPS)
        sl = slice(g * DC, (g + 1) * DC)
        in_maps.append({
            "xq": qT[b], "xk": kTc[b], "xv": vTc[b],
            "wqT": np.ascontiguousarray(W_qkv[sl, :].T).astype(np.float16),
            "wkT": np.ascontiguousarray(W_qkv[E:][sl, :].T).astype(np.float16),
            "wvT": np.ascontiguousarray(W_qkv[2 * E:][sl, :].T).astype(np.float16),
            "woT": np.ascontiguousarray(W_out[:, sl].T).astype(np.float16),
            "bq": np.ascontiguousarray(b_qkv[sl]),
            "bk": np.ascontiguousarray(b_qkv[E:][sl]),
            "bv": np.ascontiguousarray(b_qkv[2 * E:][sl]),
            "vmask": vms[b],
            "sel": sel,
        })
    return skv, in_maps


def kernel(q, k, v, mask, W_qkv, b_qkv, W_out, b_out):
    from concourse import bass_utils

    skv, in_maps = _prep_inputs(q, k, v, mask, W_qkv, b_qkv, W_out, b_out)
    if skv not in _CACHE:
        _CACHE[skv] = _build(skv)
    nc = _CACHE[skv]

    trace = os.environ.get("KERNEL_TRACE") == "1"
    if trace:
        bass_utils.upload_artifacts = lambda tmpdir: "local://" + tmpdir
    res = bass_utils.run_bass_kernel_spmd(
        nc, in_maps, list(range(NCORES)), trace=trace)
    if trace:
        global LAST_RES
        LAST_RES = res
        print(f"HW exec time: {res.exec_time_ns} ns")

    b_out = np.asarray(b_out, dtype=np.float32)
    out = np.zeros((B, S, E), np.float32)
    for c in range(NCORES):
        out[c // GROUPS] += res.results[c]["out"].astype(np.float32)
    out += b_out[None, None, :]
    return out


# revision 38
# speedup vs baseline: 1.1720x; 1.0632x over previous
"""Multi-head attention (B=2, S=2048, E=1024, H=16) on 8 TRN2 NeuronCores.

Sharding: batch x head-group. Core c handles batch b=c//4 and head group
g=c%4 (4 heads = 256 of E). Each core computes its heads' attention output
slice and a partial fc_out product [S, E]; the host sums the 4 partials per
batch and adds b_out.

Device-side math per core:
  qpT = (Wq_g @ q[b].T + bq)      [256, S]   f32r (dims on partitions)
  kpT = (Wk_g @ k_c[b].T + bk)    [256, SKV] f32r (k compressed by mask)
  vp  = (v_c[b] @ Wv_g.T + bv)*m  [SKV, 4*65] bf16 (64 dims + ones col/head)
  per (qb, pt): S_T chunks [128kv, 512q] for both heads of the pair land in
  one 2-bank psum tile -> one Exp activation [128, 1024] -> et bf16; AV
  accumulates vp_aug.T @ E_T -> [65, 512] (row 64 = softmax denominator).
  normalize: Pool copies AV psum -> bf16; a [1,64] ones matmul broadcasts
  the denominator row; DVE reciprocal + multiply -> o_bf f16.
  fc_out accumulates BOTH pairs into one psum tile -> out [S, E] f16.

The loop is qb-major with fc(qb-1) and proj_q(qb+1) matmuls interleaved
into the attention stream so the PE never idles (p-state stays ramped).

Mask handling is exact: masked K/V rows are removed on the host (gather),
so softmax(where(mask==0, -1e20, e)) == exp(e_valid)/sum(exp(e_valid)).
"""

import os

import numpy as np

B, S, E, H = 2, 2048, 1024, 16
D = E // H           # 64
NCORES = 8
GROUPS = 4           # head groups per batch (cores per batch)
HPG = H // GROUPS    # 4 heads per core
DC = E // GROUPS     # 256 dims per core
NB = E // 128        # 8 contraction chunks over E
QB = 512             # query block
NQB = S // QB        # 4

_CACHE = {}


def _split_excess_waits(nc, max_waits=1):
    """walrus rejects instructions carrying >1 sem wait; spread extras onto
    single-wait NoOps inserted before the instruction on the same engine."""
    import concourse.mybir as mybir

    n_split = 0
    for f in nc.m.functions:
        for bb in f.blocks:
            out, changed = [], False
            for ins in bb.instructions:
                si = ins.sync_info
                if si is not None and si.on_wait is not None and len(si.on_wait) > max_waits:
                    waits = list(si.on_wait)
                    for w in waits[:-max_waits]:
                        out.append(mybir.InstNoOp(
                            name=nc.get_next_instruction_name(),
                            engine=ins.engine, ins=[], outs=[],
                            sync_info=mybir.SyncInfo(on_wait=[w], on_update=[])))
                        n_split += 1
                    ins.sync_info = mybir.SyncInfo(
                        on_wait=waits[-max_waits:], on_update=list(si.on_update))
                    changed = True
                out.append(ins)
            if changed:
                bb.instructions = out
    return n_split


def _build(skv, split_waits=True):
    import concourse.bass as bass
    import concourse.mybir as mybir
    import concourse.tile as tile

    f32 = mybir.dt.float32
    f32r = mybir.dt.float32r
    f16 = mybir.dt.float16
    bf16 = mybir.dt.bfloat16
    Alu = mybir.AluOpType
    Act = mybir.ActivationFunctionType

    nsk = skv // 128

    KCH = 384 if skv % 384 == 0 else 128
    nkch = skv // KCH

    nc = bass.Bass()
    # inputs pre-packed on host so each DMA is 128 partitions x one fat
    # contiguous run (multi-KB descriptors -> near-peak HBM bandwidth)
    xq_d = nc.declare_dram_parameter("xq", [S // QB, 128, NB, QB], f16, isOutput=False)
    xk_d = nc.declare_dram_parameter("xk", [nkch, 128, NB, KCH], f16, isOutput=False)
    xv_d = nc.declare_dram_parameter("xv", [nsk, 128, NB, 128], f16, isOutput=False)
    wqT = nc.declare_dram_parameter("wqT", [E, DC], f16, isOutput=False)
    wkT = nc.declare_dram_parameter("wkT", [E, DC], f16, isOutput=False)
    wvT = nc.declare_dram_parameter("wvT", [E, DC], f16, isOutput=False)
    woT = nc.declare_dram_parameter("woT", [DC, E], f16, isOutput=False)
    bq_d = nc.declare_dram_parameter("bq", [DC], f32, isOutput=False)
    bk_d = nc.declare_dram_parameter("bk", [DC], f32, isOutput=False)
    bv_d = nc.declare_dram_parameter("bv", [DC], f32, isOutput=False)
    vm_d = nc.declare_dram_parameter("vmask", [skv], f32, isOutput=False)
    sel_d = nc.declare_dram_parameter("sel", [2, 128], bf16, isOutput=False)
    out_d = nc.declare_dram_parameter("out", [S, E], f16, isOutput=True)

    with tile.TileContext(nc) as tc:
        with (
            tc.tile_pool(name="weights", bufs=4) as wpool,
            tc.tile_pool(name="consts", bufs=1) as cpool,
            tc.tile_pool(name="persist", bufs=1) as ppool,
            tc.tile_pool(name="xq", bufs=2) as xqpool,
            tc.tile_pool(name="xkv", bufs=1) as xkvpool,
            tc.tile_pool(name="sc_ps", bufs=2, space="PSUM") as scps,
            tc.tile_pool(name="av_ps", bufs=2, space="PSUM") as avps,
            tc.tile_pool(name="fc_ps", bufs=2, space="PSUM") as fps,
            tc.tile_pool(name="et", bufs=3) as etpool,
            tc.tile_pool(name="ou", bufs=3) as oupool,
            tc.tile_pool(name="rec", bufs=3) as rcpool,
            tc.tile_pool(name="outp", bufs=2) as opool,
            tc.tile_pool(name="small", bufs=2) as smpool,
        ):
            # ---- constants / weights. All big inputs go on the sync queue
            # in priority order (its FIFO then orders the transfers on the
            # bandwidth-saturated DMA engines); consts on gpsimd; scalar
            # queue stays pure-exp. k first: kpT gates attention.
            wk_t = wpool.tile([128, NB, DC], f16, tag="w", name="wk_t")
            wq_t = wpool.tile([128, NB, DC], f16, tag="w", name="wq_t")
            wv_t = wpool.tile([128, NB, DC], f16, tag="w", name="wv_t")
            wo_t = wpool.tile([128, DC // 128, E], f16, tag="w", name="wo_t")
            nc.sync.dma_start(wk_t[:], wkT.rearrange("(ko p) m -> p ko m", p=128))

            xk = xkvpool.tile([128, nkch, NB, KCH], f16, tag="xk", name="xk")
            xv = xkvpool.tile([128, nsk, NB, 128], f16, tag="xv", name="xv")
            for kb in range(nkch):
                nc.sync.dma_start(xk[:, kb, :, :], xk_d[kb])

            xq = [xqpool.tile([128, NB, QB], f16, tag="xq", name=f"xq{nb}")
                  for nb in range(NQB)]
            nc.sync.dma_start(wq_t[:], wqT.rearrange("(ko p) m -> p ko m", p=128))
            nc.sync.dma_start(xq[0][:], xq_d[0])
            nc.sync.dma_start(wv_t[:], wvT.rearrange("(ko p) m -> p ko m", p=128))
            for sc in range(nsk):
                nc.sync.dma_start(xv[:, sc, :, :], xv_d[sc])
            for nb in range(1, NQB):
                nc.sync.dma_start(xq[nb][:], xq_d[nb])
            nc.sync.dma_start(wo_t[:], woT.rearrange("(ko p) n -> p ko n", p=128))

            bq_t = cpool.tile([128, 2], f32, tag="bq")
            bk_t = cpool.tile([128, 2], f32, tag="bk")
            bv_t = cpool.tile([128, DC], f32, tag="bv")
            vm_t = cpool.tile([128, nsk], f32, tag="vm")
            nc.gpsimd.dma_start(bk_t[:], bk_d.rearrange("(c p) -> p c", p=128))
            nc.gpsimd.dma_start(bq_t[:], bq_d.rearrange("(c p) -> p c", p=128))
            nc.gpsimd.dma_start(bv_t[:], bv_d[None, :].to_broadcast((128, DC)))
            nc.gpsimd.dma_start(vm_t[:], vm_d.rearrange("(s p) -> p s", p=128))

            # selection matrix: sel[k, p] = 1 iff p // 64 == k; broadcasts
            # rr row j to output partitions 64j..64j+63 in one matmul
            sel_t = cpool.tile([2, 128], bf16, tag="sel")
            nc.gpsimd.dma_start(sel_t[:], sel_d[:])

            qpT = ppool.tile([128, 2, S], f32r, tag="qpT")
            kpT = ppool.tile([128, 2, skv], f32r, tag="kpT")
            vp = ppool.tile([128, nsk, HPG, D + 1], bf16, tag="vp")
            o_bf = ppool.tile([128, 2, S], f16, tag="o_bf")

            # ---- projection emitters ----
            def proj_k(kb):
                off = kb * KCH
                for mc in range(2):
                    ps = fps.tile([128, 512], f32, tag="fc", name="kp_ps")[:, :KCH]
                    for kc in range(NB):
                        nc.tensor.matmul(
                            ps[:], wk_t[:, kc, mc * 128:(mc + 1) * 128],
                            xk[:, kb, kc, :],
                            start=(kc == 0), stop=(kc == NB - 1))
                    nc.vector.tensor_tensor(
                        out=kpT[:, mc, off:off + KCH], in0=ps[:],
                        in1=bk_t[:, mc:mc + 1].to_broadcast((128, KCH)), op=Alu.add)

            def proj_q_mms(nb, mc, ps):
                """8 accumulating matmuls for qpT block nb, half mc."""
                for kc in range(NB):
                    yield lambda kc=kc: nc.tensor.matmul(
                        ps[:], wq_t[:, kc, mc * 128:(mc + 1) * 128],
                        xq[nb][:, kc, :], start=(kc == 0), stop=(kc == NB - 1))

            def proj_q_bias(nb, mc, ps):
                nc.vector.tensor_tensor(
                    out=qpT[:, mc, nb * QB:(nb + 1) * QB], in0=ps[:],
                    in1=bq_t[:, mc:mc + 1].to_broadcast((128, QB)), op=Alu.add)

            def proj_q(nb):
                for mc in range(2):
                    ps = fps.tile([128, 512], f32, tag="fc", name="qp_ps")
                    for mm in proj_q_mms(nb, mc, ps):
                        mm()
                    proj_q_bias(nb, mc, ps)

            def proj_v(sc):
                ps = fps.tile([128, 512], f32, tag="fc", name="vp_ps")[:, :DC]
                for kc in range(NB):
                    nc.tensor.matmul(
                        ps[:], xv[:, sc, kc, :], wv_t[:, kc, :],
                        start=(kc == 0), stop=(kc == NB - 1))
                t1 = smpool.tile([128, DC], f32, tag="vtmp")
                nc.vector.tensor_tensor(out=t1[:], in0=ps[:], in1=bv_t[:], op=Alu.add)
                nc.gpsimd.tensor_tensor(
                    out=vp[:, sc, :, 0:D],
                    in0=t1.rearrange("p (h w) -> p h w", w=D),
                    in1=vm_t[:, sc:sc + 1, None].to_broadcast((128, HPG, D)),
                    op=Alu.mult)
                nc.gpsimd.tensor_copy(
                    out=vp[:, sc, :, D:D + 1],
                    in_=vm_t[:, sc:sc + 1, None].to_broadcast((128, HPG, 1)))

            # ---- lead-in: kpT, qpT block 0 (vp is projected just-in-time
            # inside qb0/pt0's chunk loop as xv chunks stream in) ----
            for kb in range(nkch):
                proj_k(kb)
            proj_q(0)

            # ---- fc_out emitter (query row sqc: 128 queries, eb: 512 cols) --
            def fc_mms(sqc, eb, ps):
                for pt in range(2):
                    yield lambda pt=pt: nc.tensor.matmul(
                        ps[:], o_bf[:, pt, sqc * 128:(sqc + 1) * 128],
                        wo_t[:, pt, eb * 512:(eb + 1) * 512],
                        start=(pt == 0), stop=(pt == 1))

            def fc_tail(sqc, eb, ps, ob, last=False):
                nc.vector.tensor_copy(
                    out=ob[:, eb * 512:(eb + 1) * 512], in_=ps[:])
                if eb == 1:
                    nc.sync.dma_start(out_d[sqc * 128:(sqc + 1) * 128, :], ob[:])

            def fc_block(qb, last=False):
                """All fc work for query block qb as a list of thunks
                (each one PE matmul or a tail), to interleave."""
                thunks = []
                for sq in range(QB // 128):
                    sqc = qb * (QB // 128) + sq
                    ob = opool.tile([128, E], f16, tag="ob", name="ob")
                    for eb in range(2):
                        ps = fps.tile([128, 512], f32, tag="fc", name="fc_ps")
                        for mm in fc_mms(sqc, eb, ps):
                            thunks.append(mm)
                        thunks.append(
                            lambda sqc=sqc, eb=eb, ps=ps, ob=ob:
                                fc_tail(sqc, eb, ps, ob, last))
                return thunks

            def block_finish(qb, pt, ps_av, et_last):
                """Thunks that finish block (qb, pt): last AV chunk, then
                normalize (AV psum -> bf16, denominator rows DMA-reshaped to
                [128, 8] for a cheap reciprocal, broadcast back via the
                selection matmul). Run after the NEXT block's first scores."""
                q0 = qb * QB
                # the very last block routes its reshape DMAs over the
                # then-idle sync queue to shorten the tail latency chain
                dq = nc.sync if (qb == NQB - 1 and pt == 1) else nc.gpsimd
                thunks = [
                    (lambda j=j: nc.tensor.matmul(
                        ps_av[j][:], vp[:, nsk - 1, 2 * pt + j, :],
                        et_last[:, j, :], start=False, stop=True))
                    for j in range(2)
                ]

                def norm():
                    ous = []
                    scol = rcpool.tile([128, 8], bf16, tag="scol", name="scol")
                    for j in range(2):
                        ou = oupool.tile([D + 1, QB], bf16, tag="ou", name="ou")
                        nc.vector.tensor_copy(out=ou[:], in_=ps_av[j][:])
                        ous.append(ou)
                        # denom row q=p*4+c -> scol[p, j*4+c]
                        dq.dma_start(
                            scol[:, j * 4:(j + 1) * 4],
                            ou[64:65, :].rearrange("o (p c) -> o p c", c=4))
                    with nc.allow_low_precision(reason="softmax denom recip"):
                        nc.vector.reciprocal(out=scol[:], in_=scol[:])
                    rr = rcpool.tile([2, QB], bf16, tag="rr", name="rr")
                    # scol[p, j*4+c] -> rr[j, p*4+c], iterating (p, c)
                    for j in range(2):
                        dq.dma_start(
                            rr[j:j + 1, :].rearrange("o (p c) -> o p c", c=4),
                            scol[:, j * 4:(j + 1) * 4])
                    bc = fps.tile([128, 512], f32, tag="fc", name="bc")
                    nc.tensor.matmul(bc[:], sel_t[:], rr[:],
                                     start=True, stop=True)
                    for j in range(2):
                        nc.vector.tensor_tensor(
                            out=o_bf[64 * j:64 * j + 64, pt, q0:q0 + QB],
                            in0=ous[j][0:D, :],
                            in1=bc[64 * j:64 * j + 64, :], op=Alu.mult)

                thunks.append(norm)
                return thunks

            # ---- attention ----
            pending = []
            for qb in range(NQB):
                q0 = qb * QB
                # interleavable PE work: fc of previous block + proj of next q
                extra = fc_block(qb - 1) if qb > 0 else []
                if qb + 1 < NQB:
                    for mc in range(2):
                        ps = fps.tile([128, 512], f32, tag="fc", name="qp_ps")
                        for mm in proj_q_mms(qb + 1, mc, ps):
                            extra.append(mm)
                        extra.append(
                            lambda nb=qb + 1, mc=mc, ps=ps: proj_q_bias(nb, mc, ps))
                ei = 0

                def drain(n):
                    nonlocal ei
                    for _ in range(n):
                        if ei < len(extra):
                            extra[ei]()
                            ei += 1

                for pt in range(2):
                    ps_av = [None, None]
                    ets = []
                    for skc in range(nsk):
                        psx = scps.tile([128, 2, QB], f32, tag="sc", name="psx")
                        for j in range(2):
                            nc.tensor.matmul(
                                psx[:, j, :],
                                kpT[64 * j:64 * j + 64, pt, skc * 128:(skc + 1) * 128],
                                qpT[64 * j:64 * j + 64, pt, q0:q0 + QB],
                                start=True, stop=True, tile_position=(64 * j, 0))
                        if skc == 0:
                            # finish the previous block (its last AV +
                            # normalize) AFTER this block's first scores so
                            # the exp stream never drains at the boundary
                            for th in pending:
                                th()
                            pending.clear()
                            for j in range(2):
                                ps_av[j] = avps.tile(
                                    [D + 1, QB], f32, tag="av", name=f"av{j}")
                        if qb == 0 and pt == 0:
                            proj_v(skc)  # just-in-time: AV(skc) runs next step
                        et = etpool.tile([128, 2, QB], bf16, tag="et", name="et")
                        nc.scalar.activation(et[:], psx[:], Act.Exp)
                        ets.append(et)
                        # AV for previous chunk (keeps one step of slack
                        # between PE and the exp on the scalar engine)
                        if skc > 0:
                            for j in range(2):
                                nc.tensor.matmul(
                                    ps_av[j][:], vp[:, skc - 1, 2 * pt + j, :],
                                    ets[skc - 1][:, j, :],
                                    start=(skc - 1 == 0), stop=False)
                        if not (qb == 0 and pt == 0) and skc >= 2:
                            drain(2)
                    pending.extend(block_finish(qb, pt, ps_av, ets[nsk - 1]))
                # any leftover interleaved work
                drain(len(extra))
            for th in pending:
                th()
            pending.clear()
            # final fc block
            for th in fc_block(NQB - 1, last=True):
                th()

    if split_waits:
        _split_excess_waits(nc)
    return nc


def _prep_inputs(q, k, v, mask, W_qkv, b_qkv, W_out, b_out):
    """Host-side shard/layout prep. Returns (skv, in_maps)."""
    q = np.asarray(q, dtype=np.float32)
    k = np.asarray(k, dtype=np.float32)
    v = np.asarray(v, dtype=np.float32)
    mask = np.asarray(mask)
    W_qkv = np.asarray(W_qkv, dtype=np.float32)
    b_qkv = np.asarray(b_qkv, dtype=np.float32)
    W_out = np.asarray(W_out, dtype=np.float32)

    valid = [np.nonzero(mask[b, 0, 0] != 0)[0] for b in range(B)]
    cnts = [len(vi) for vi in valid]
    skv = max(128, max((c + 127) // 128 * 128 for c in cnts))
    KCH = 384 if skv % 384 == 0 else 128
    nkch = skv // KCH
    nsk = skv // 128

    # pack [E, n] tensors as [chunk, 128, NB, w]: per-(chunk, partition)
    # contiguous runs so each device DMA is 128 fat descriptors
    def pack(tT, w):
        n = tT.shape[1]
        return np.ascontiguousarray(
            tT.reshape(NB, 128, n // w, w).transpose(2, 1, 0, 3))

    qT, kTc, vTc, vms = [], [], [], []
    for b in range(B):
        qT.append(pack(q[b].T.astype(np.float16), 512))
        kt = np.zeros((E, skv), np.float16)
        vt = np.zeros((E, skv), np.float16)
        kt[:, :cnts[b]] = k[b][valid[b]].T
        vt[:, :cnts[b]] = v[b][valid[b]].T
        kTc.append(pack(kt, KCH))
        vTc.append(pack(vt, 128))
        vm = np.zeros((skv,), np.float32)
        vm[:cnts[b]] = 1.0
        vms.append(vm)

    import ml_dtypes
    sel = np.zeros((2, 128), dtype=ml_dtypes.bfloat16)
    sel[0, 0:64] = 1
    sel[1, 64:128] = 1

    in_maps = []
    for c in range(NCORES):
        b, g = divmod(c, GROUPS)
        sl = slice(g * DC, (g + 1) * DC)
        in_maps.append({
            "xq": qT[b], "xk": kTc[b], "xv": vTc[b],
            "wqT": np.ascontiguousarray(W_qkv[sl, :].T).astype(np.float16),
            "wkT": np.ascontiguousarray(W_qkv[E:][sl, :].T).astype(np.float16),
            "wvT": np.ascontiguousarray(W_qkv[2 * E:][sl, :].T).astype(np.float16),
            "woT": np.ascontiguousarray(W_out[:, sl].T).astype(np.float16),
            "bq": np.ascontiguousarray(b_qkv[sl]),
            "bk": np.ascontiguousarray(b_qkv[E:][sl]),
            "bv": np.ascontiguousarray(b_qkv[2 * E:][sl]),
            "vmask": vms[b],
            "sel": sel,
        })
    return skv, in_maps


def kernel(q, k, v, mask, W_qkv, b_qkv, W_out, b_out):
    from concourse import bass_utils

    skv, in_maps = _prep_inputs(q, k, v, mask, W_qkv, b_qkv, W_out, b_out)
    if skv not in _CACHE:
        _CACHE[skv] = _build(skv)
    nc = _CACHE[skv]

    trace = os.environ.get("KERNEL_TRACE") == "1"
    if trace:
        bass_utils.upload_artifacts = lambda tmpdir: "local://" + tmpdir
    res = bass_utils.run_bass_kernel_spmd(
        nc, in_maps, list(range(NCORES)), trace=trace)
    if trace:
        global LAST_RES
        LAST_RES = res
        print(f"HW exec time: {res.exec_time_ns} ns")

    b_out = np.asarray(b_out, dtype=np.float32)
    out = np.zeros((B, S, E), np.float32)
    for c in range(NCORES):
        out[c // GROUPS] += res.results[c]["out"].astype(np.float32)
    out += b_out[None, None, :]
    return out
